# revision 20
# baseline (speedup 1.0000x reference)
"""Multi-head causal self-attention (B=2, S=2048, D=1024, H=16) on 8 NeuronCores.

Sharding: core c handles batch b = c // 4 and heads 4j..4j+3 where j = c % 4
(tensor-parallel over heads within a 4-core group, data-parallel over batch
across the two groups).  Each core:
  1. loads x[b]^T and its column slices of Wq/Wk/Wv,
  2. computes Q^T/K^T (feature-major) and V (seq-major) for its 4 heads,
  3. runs causal softmax attention per head entirely on-chip
     (scores are computed transposed, S^T[k, q], so no transposes are needed;
      the softmax denominator comes from an appended ones-column in V),
  4. AllGathers the normalized ctx^T across its 4-core group,
  5. computes its 256-column slice of the output projection (+bias).
The host assembles the 8 disjoint output slices.

Heads are processed in pairs that share the 128 partitions (rows 0-63 = even
head, 64-127 = odd head), so the two S^T matmuls of a pair run concurrently
in distinct PE row groups and the exp over both heads is a single fused
activation over a 2-bank PSUM tile.  The attention inner loop is emitted
software-pipelined: the S^T matmul of k-tile ki+1 is issued before the PV
matmul of k-tile ki so the PE never head-of-line blocks on the exp.
"""

import math

import numpy as np

import concourse.bass as bass
import concourse.tile as tile
from concourse import bacc, mybir
from concourse.bass_utils import run_bass_kernel_spmd

B, S, D, H, DH = 2, 2048, 1024, 16, 64
NCORES = 8
GROUP = 4          # cores per batch group
HPC = 4            # heads per core
FPC = HPC * DH     # 256 features per core
QB = 512           # q block width (matmul moving free dim)
KT = 128           # k tile height (partition dim)
SCALE = 1.0 / math.sqrt(S)

F32 = mybir.dt.float32
F32R = mybir.dt.float32r
EXP = mybir.ActivationFunctionType.Exp


def build_program(sim_collective=False, reps=1):
    """sim_collective=True replaces the AllGather with equivalent-volume local
    DMA traffic so the (single-core) TimelineSim cost model can run.
    reps>1 repeats the whole body inside one NEFF (for slope timing)."""
    nc = bacc.Bacc(
        "TRN2",
        target_bir_lowering=False,
        debug=False,
        num_devices=NCORES,
    )

    xT = nc.dram_tensor("xT", [D, S], F32R, kind="ExternalInput").ap()
    wq = nc.dram_tensor("wq", [D, FPC], F32R, kind="ExternalInput").ap()
    wk = nc.dram_tensor("wk", [D, FPC], F32R, kind="ExternalInput").ap()
    wv = nc.dram_tensor("wv", [D, FPC], F32R, kind="ExternalInput").ap()
    wo = nc.dram_tensor("wo", [D, FPC], F32R, kind="ExternalInput").ap()
    bo = nc.dram_tensor("bo", [1, FPC], F32, kind="ExternalInput").ap()
    tri = nc.dram_tensor("tri", [KT, 2 * KT], F32, kind="ExternalInput").ap()
    ones = nc.dram_tensor("ones", [128, 16 * HPC], F32R, kind="ExternalInput").ap()
    out = nc.dram_tensor("out", [S, FPC], F32, kind="ExternalOutput").ap()

    with tile.TileContext(nc) as tc:
      for _rep in range(reps):
        with (
            tc.tile_pool(name="cpool", bufs=1) as cpool,
            tc.tile_pool(name="qkvp", bufs=1) as qkvp,
            tc.tile_pool(name="dpool", bufs=1, space="DRAM") as dpool,
        ):
            # ---- constants / weights -------------------------------------
            wq_sb = cpool.tile([128, 8, FPC], F32R)
            wk_sb = cpool.tile([128, 8, FPC], F32R)
            wv_sb = cpool.tile([128, 8, FPC], F32R)
            wo_sb = cpool.tile([128, 8, FPC], F32R)
            nc.sync.dma_start(wv_sb[:], wv.rearrange("(t p) f -> p t f", p=128))

            # ---- persistent per-core tensors -----------------------------
            qT_sb = qkvp.tile([128, 2, S], F32R)   # [dh, head-pair, seq]
            kT_sb = qkvp.tile([128, 2, S], F32R)
            v_sb = qkvp.tile([128, 16, HPC * (DH + 1)], F32R)  # [k, seqtile, 4*65]
            v4 = v_sb.rearrange("p s (h e) -> p s h e", e=DH + 1)

            cc_in0 = dpool.tile([2 * DH, S], F32R)
            cc_in1a = dpool.tile([2 * DH, S // 2], F32R)
            cc_in1b = dpool.tile([2 * DH, S // 2], F32R)
            cc_out0 = dpool.tile([GROUP * 2 * DH, S], F32R)
            cc_out1a = dpool.tile([GROUP * 2 * DH, S // 2], F32R)
            cc_out1b = dpool.tile([GROUP * 2 * DH, S // 2], F32R)

            # ---- projections ---------------------------------------------
            with (
                tc.tile_pool(name="xtp", bufs=1) as xtp,
                tc.tile_pool(name="pjp", bufs=2, space="PSUM") as pjp,
            ):
                xt_sb = xtp.tile([128, 8, S], F32R)
                xt_dram = xT.rearrange("(t p) m -> t p m", p=128)
                for c in range(4):
                    cs = slice(c * S // 4, (c + 1) * S // 4)
                    for t in range(8):
                        nc.sync.dma_start(xt_sb[:, t, cs], xt_dram[t][:, cs])

                # remaining constants, after the latency-critical wv/xT loads
                nc.sync.dma_start(wk_sb[:], wk.rearrange("(t p) f -> p t f", p=128))
                nc.sync.dma_start(wq_sb[:], wq.rearrange("(t p) f -> p t f", p=128))
                nc.sync.dma_start(
                    v4[:, :, :, DH], ones.rearrange("p (s h) -> p s h", h=HPC)
                )
                tri_sb = cpool.tile([KT, 2, KT], F32)
                nc.sync.dma_start(tri_sb[:], tri.rearrange("p (h q) -> p h q", q=KT))
                bo_sb = cpool.tile([1, FPC], F32)
                nc.sync.dma_start(bo_sb[:], bo)
                bias_bc = cpool.tile([128, FPC], F32)
                nc.gpsimd.partition_broadcast(bias_bc[:], bo_sb[:])
                nc.sync.dma_start(wo_sb[:], wo.rearrange("(t p) f -> p t f", p=128))

                # V: seq-major, heads interleaved with ones column (emitted
                # first: the attention k-loop consumes V tiles in order)
                for s in range(16):
                    ps = pjp.tile([128, FPC], F32, tag="pjv", bufs=6)
                    for t in range(8):
                        nc.tensor.matmul(
                            ps[:],
                            xt_sb[:, t, s * 128:(s + 1) * 128],
                            wv_sb[:, t],
                            start=(t == 0),
                            stop=(t == 7),
                        )
                    nc.vector.tensor_copy(
                        v4[:, s, :, 0:DH],
                        ps.rearrange("p (h e) -> p h e", e=DH),
                    )

                # K^T / Q^T per head pair (pair 0 first so attention on the
                # first pair can begin while the second pair projects)
                for f in range(2):
                    for w_sb, dst in ((wk_sb, kT_sb), (wq_sb, qT_sb)):
                        for qb in range(4):
                            ps = pjp.tile([128, QB], F32, tag="pj")
                            for t in range(8):
                                nc.tensor.matmul(
                                    ps[:],
                                    w_sb[:, t, f * 128:(f + 1) * 128],
                                    xt_sb[:, t, qb * QB:(qb + 1) * QB],
                                    start=(t == 0),
                                    stop=(t == 7),
                                )
                            nc.vector.tensor_copy(
                                dst[:, f, qb * QB:(qb + 1) * QB], ps[:]
                            )

            # ---- attention ------------------------------------------------
            # ogp spans attention AND the output projection: the gathered
            # ctx^T tiles are loaded as soon as each pair's AllGather lands
            ogp = tc.tile_pool(name="ogp", bufs=1)
            ogp_pool = ogp.__enter__()
            ctxg = ogp_pool.tile([128, 8, S], F32R)
            ccg0 = cc_out0.rearrange("(f p) q -> f p q", p=128)
            ccg1a = cc_out1a.rearrange("(f p) q -> f p q", p=128)
            ccg1b = cc_out1b.rearrange("(f p) q -> f p q", p=128)
            with (
                tc.tile_pool(name="attp", bufs=6) as attp,
                tc.tile_pool(name="stp", bufs=2, space="PSUM") as stp,
                tc.tile_pool(name="ctxp", bufs=4, space="PSUM") as ctxp,
                tc.tile_pool(name="nrmp", bufs=4) as nrmp,
            ):
                for qb, pair in [(q, p) for p in range(2) for q in range(4)]:
                    if True:
                        h0, h1 = 2 * pair, 2 * pair + 1
                        nk = 4 * (qb + 1)
                        qs = slice(qb * QB, (qb + 1) * QB)
                        ctx0 = ctxp.tile([DH + 1, QB], F32, tag="ctx",
                                         name=f"ctx0_{pair}_{qb}")
                        ctx1 = ctxp.tile([DH + 1, QB], F32, tag="ctx",
                                         name=f"ctx1_{pair}_{qb}")

                        sts = [None] * nk
                        pts = [None] * nk

                        def emit_s(ki):
                            ks = slice(ki * KT, (ki + 1) * KT)
                            st = stp.tile([128, 2 * QB], F32, tag="st",
                                          name=f"st_{pair}_{qb}_{ki}")
                            nc.tensor.matmul(
                                st[:, 0:QB], kT_sb[0:64, pair, ks],
                                qT_sb[0:64, pair, qs], start=True, stop=True,
                            )
                            nc.tensor.matmul(
                                st[:, QB:2 * QB], kT_sb[64:128, pair, ks],
                                qT_sb[64:128, pair, qs], start=True, stop=True,
                            )
                            sts[ki] = st

                        def emit_exp(ki):
                            st2 = sts[ki].rearrange("p (h q) -> p h q", q=QB)
                            pt = attp.tile([128, 2, QB], F32R, tag="pt",
                                           name=f"pt_{pair}_{qb}_{ki}")
                            off = ki * KT - qb * QB
                            if off <= 0:
                                nc.scalar.activation(pt[:], st2[:], EXP,
                                                     scale=SCALE)
                            else:
                                # cols < off are fully masked: never computed,
                                # and the PV matmul skips them below
                                nc.scalar.activation(
                                    pt[:, :, off:], st2[:, :, off:], EXP,
                                    scale=SCALE,
                                )
                            if off >= 0:
                                nc.vector.tensor_mul(
                                    pt[:, :, off:off + KT],
                                    pt[:, :, off:off + KT],
                                    tri_sb[:],
                                )
                            pts[ki] = pt

                        def emit_pv(ki):
                            pt = pts[ki]
                            off = max(ki * KT - qb * QB, 0)
                            nc.tensor.matmul(
                                ctx0[:, off:], v4[:, ki, h0], pt[:, 0, off:],
                                start=(ki == 0), stop=(ki == nk - 1),
                            )
                            nc.tensor.matmul(
                                ctx1[:, off:], v4[:, ki, h1], pt[:, 1, off:],
                                start=(ki == 0), stop=(ki == nk - 1),
                            )

                        emit_s(0)
                        emit_exp(0)
                        for ki in range(nk):
                            if ki + 1 < nk:
                                emit_s(ki + 1)
                            emit_pv(ki)
                            if ki + 1 < nk:
                                emit_exp(ki + 1)

                        for h, ctx in ((h0, ctx0), (h1, ctx1)):
                            rc = nrmp.tile([1, QB], F32, tag="rc",
                                           name=f"rc_{pair}_{qb}_{h}")
                            nc.vector.reciprocal(rc[:], ctx[DH:DH + 1, :])
                            bc = nrmp.tile([64, QB], F32, tag="bc",
                                           name=f"bc_{pair}_{qb}_{h}")
                            nc.gpsimd.partition_broadcast(bc[:], rc[:])
                            cn = nrmp.tile([64, QB], F32R, tag="cn",
                                           name=f"cn_{pair}_{qb}_{h}")
                            nc.vector.tensor_mul(cn[:], ctx[0:DH, :], bc[:])
                            row = slice((h % 2) * DH, (h % 2 + 1) * DH)
                            if pair == 0:
                                nc.sync.dma_start(cc_in0[row, qs], cn[:])
                            elif qb < 2:
                                nc.sync.dma_start(cc_in1a[row, qs], cn[:])
                            else:
                                nc.sync.dma_start(
                                    cc_in1b[row, qb * QB - S // 2:
                                            (qb + 1) * QB - S // 2], cn[:])

                        def gather(cin, cout, ctxg_dst, ccg):
                            if sim_collective:
                                for g in range(GROUP):
                                    nc.sync.dma_start(
                                        cout[g * 2 * DH:(g + 1) * 2 * DH, :],
                                        cin[:],
                                    )
                            else:
                                nc.gpsimd.collective_compute(
                                    "AllGather",
                                    mybir.AluOpType.bypass,
                                    replica_groups=[[0, 1, 2, 3], [4, 5, 6, 7]],
                                    ins=[cin.opt()],
                                    outs=[cout.opt()],
                                )
                            for f in range(4):
                                nc.sync.dma_start(ctxg_dst(f), ccg[f])

                        if pair == 0 and qb == 3:
                            gather(cc_in0, cc_out0,
                                   lambda f: ctxg[:, f], ccg0)
                        elif pair == 1 and qb == 1:
                            gather(cc_in1a, cc_out1a,
                                   lambda f: ctxg[:, 4 + f, 0:S // 2], ccg1a)
                        elif pair == 1 and qb == 3:
                            gather(cc_in1b, cc_out1b,
                                   lambda f: ctxg[:, 4 + f, S // 2:], ccg1b)


            # ---- output projection ---------------------------------------
            with (
                tc.tile_pool(name="opp", bufs=8, space="PSUM") as opp,
                tc.tile_pool(name="obp", bufs=3) as obp,
            ):
                # Phase A: q-tiles 0..7 against the pair-0 gather (f 0..3),
                # f-major so these 32 matmuls run while the pair-1 gather is
                # still in flight.  One PSUM bank per live q-tile.
                pss = {}
                for s in range(8):
                    pss[s] = opp.tile([128, FPC], F32, tag="op",
                                      name=f"ops_{s}")
                for f in range(4):
                    for s in range(8):
                        nc.tensor.matmul(
                            pss[s][:],
                            ctxg[:, f, s * 128:(s + 1) * 128],
                            wo_sb[:, f],
                            start=(f == 0),
                            stop=False,
                        )

                def finish_tile(s, ps, f0):
                    for f in range(f0, 8):
                        nc.tensor.matmul(
                            ps[:],
                            ctxg[:, f, s * 128:(s + 1) * 128],
                            wo_sb[:, f],
                            start=(f == 0),
                            stop=(f == 7),
                        )
                    ot = obp.tile([128, FPC], F32, tag="ot", name=f"ot_{s}")
                    nc.vector.tensor_add(ot[:], ps[:], bias_bc[:])
                    nc.sync.dma_start(out[s * 128:(s + 1) * 128, :], ot[:])

                # Phase B: finish q-tiles 0..7 (f 4..7 + bias + store)
                for s in range(8):
                    finish_tile(s, pss[s], 4)
                # Phase C: q-tiles 8..15 start to finish
                for s in range(8, 16):
                    ps = opp.tile([128, FPC], F32, tag="op", name=f"ops_{s}")
                    finish_tile(s, ps, 0)

            ogp.__exit__(None, None, None)

    nc.compile()
    return nc


_PROGRAM = None


def _get_program():
    global _PROGRAM
    if _PROGRAM is None:
        _PROGRAM = build_program()
    return _PROGRAM


def _make_tri():
    # tri[i, j] = 1 where key-offset i <= query-offset j (allowed); the two
    # copies along the free dim serve the two heads of a fused pair tile
    i = np.arange(KT)[:, None]
    j = np.arange(KT)[None, :]
    t = (i <= j).astype(np.float32)
    return np.concatenate([t, t], axis=1)


def make_in_maps(x, Wq, Wk, Wv, Wo, bo):
    tri_arr = _make_tri()
    ones_arr = np.ones((128, 16 * HPC), np.float32)
    xTs = [np.ascontiguousarray(x[b].T) for b in range(B)]
    # Wo rows permuted to match the gathered ctx^T feature order:
    # gather0 rows = (rank j, heads 4j+0, 4j+1), gather1 = (rank j, 4j+2, 4j+3)
    perm = [4 * j + p for g in range(2) for j in range(GROUP)
            for p in (2 * g, 2 * g + 1)]
    Wo_perm = Wo.reshape(H, DH, D)[perm].reshape(D, D)
    in_maps = []
    for c in range(NCORES):
        b, j = divmod(c, GROUP)
        cols = slice(FPC * j, FPC * (j + 1))
        in_maps.append({
            "xT": xTs[b],
            "wq": np.ascontiguousarray(Wq[:, cols]),
            "wk": np.ascontiguousarray(Wk[:, cols]),
            "wv": np.ascontiguousarray(Wv[:, cols]),
            "wo": np.ascontiguousarray(Wo_perm[:, cols]),
            "bo": np.ascontiguousarray(bo[cols][None, :]),
            "tri": tri_arr,
            "ones": ones_arr,
        })
    return in_maps


def kernel(x, Wq, Wk, Wv, Wo, bo):
    x = np.ascontiguousarray(np.asarray(x, np.float32))
    Wq = np.asarray(Wq, np.float32)
    Wk = np.asarray(Wk, np.float32)
    Wv = np.asarray(Wv, np.float32)
    Wo = np.asarray(Wo, np.float32)
    bo = np.asarray(bo, np.float32)

    in_maps = make_in_maps(x, Wq, Wk, Wv, Wo, bo)
    nc = _get_program()
    results = run_bass_kernel_spmd(nc, in_maps, list(range(NCORES))).results

    out = np.empty((B, S, D), np.float32)
    for c in range(NCORES):
        b, j = divmod(c, GROUP)
        out[b, :, FPC * j:FPC * (j + 1)] = results[c]["out"]
    return out
